# revision 37
# baseline (speedup 1.0000x reference)
"""AddShift_mp_linear_module on 8 TRN2 NeuronCores — final.

Strategy (channel-block sharding, no collectives):
  - 96 output-channel blocks (11 input channels each) -> 12 blocks/core.
  - Per block, three dense contractions, all with M=56 output columns:
      V:  out_v[h, (b,w)]  over (k,h') rows   (h-major x)
      H:  out_hT[w, (b,h)] over (k,w') rows   (w-major x)
      I:  out_i[h, (b,w)]  reads 56-row slices of the SAME h-major x
    TWO chains run CONCURRENTLY on the PE via column tiling
    (tile_position (0,0)/(0,64)): measured pair span == single span
    (~190 ns warm, N=448), so PE work hides under the DMA stream.
  - Row layout per block: 11 channel groups of 64 rows (60 real + 4 pad),
    identity channels first with spatial order [2..57, 0, 1, 58, 59], so
    each identity window is a 0/64-aligned 56-row run readable straight
    from the main x tiles (no separate identity gather shipped). Identity
    = 2 K=120 matmuls/block with zero operator rows over the gaps.
  - 704 rows = 5x128 chunks + 64-row leftover; the two leftover groups of
    a block PAIR pack into one 128-partition chunk, consumed by per-block
    stationaries that are zero over the other block's rows. EVERY
    transfer is 128-partition (all 16 SDMA engines) -> ~410 GB/s.
  - DMA: ONE HWDGE ring (sync) in consumption order: per-pair operator
    slice, two x blocks, leftover chunk; outputs flush at the end from
    per-pair staging. ~10.7 MB/core total (the memory roofline).
  - Precision: fp8 e3m4 in, bf16 out (rel_err 1.85e-2 < 2e-2 gate).
Measured: ~41 us median (baseline 96.3 us).
"""

import numpy as np
import ml_dtypes

# architecture constants (match reference init_kwargs)
B = 8
C_OUT = 96
NK = 11
G = 4
C_IN = C_OUT * NK          # 1056
HOUT = WOUT = 56
HIN = WIN = 60
EP = 2                     # extra pad
N_CORES = 8
BPC = C_OUT // N_CORES     # blocks per core = 12
CPC = BPC * NK             # channels per core = 132
NPAIR = BPC // 2           # block pairs per core = 6
KG = 64                    # padded rows per channel group
KROWS = NK * KG            # 704 padded contraction rows per block
KM = 128                   # main chunk rows
NJM = 5                    # main chunks (5 x 128 = 640 rows = 10 groups)
KT = KG                    # tail chunk rows = 64 (the 11th group)
NFREE = B * WOUT           # 448 matmul free dim
N_WARM = 16                # PE warmup matmuls while first DMAs fly
OPW = NJM * 112 + 2 * 56 + 112  # operator cols per block = 784

F8 = ml_dtypes.float8_e3m4

_CACHE = {}


def _build_vh_operators(w1, w2, pad_hv):
    """Dense V/H operators per block: (C_OUT, 660, 56) fp32 each.
    Row r = k*60 + spatial_in for channel c = co*11 + k."""
    w1r = np.asarray(w1, np.float32).reshape(G, C_IN)
    w2r = np.asarray(w2, np.float32).reshape(G, C_IN)
    pad = np.asarray(pad_hv, np.int64)            # (C_IN, 2G)
    opv = np.zeros((C_OUT, NK * HIN, 56), np.float32)
    oph = np.zeros((C_OUT, NK * HIN, 56), np.float32)
    c_all = np.arange(C_IN)
    co_all = c_all // NK
    k_all = c_all % NK
    pos = np.arange(HOUT)
    for g in range(G):
        win = pos[None, :] + EP + pad[:, g][:, None]        # (C_IN, 56)
        ok = (win >= 0) & (win < WIN)
        cc, oo = np.nonzero(ok)
        np.add.at(oph, (co_all[cc], k_all[cc] * HIN + win[cc, oo], oo), w1r[g, cc])
        hin = pos[None, :] + EP + pad[:, G + g][:, None]
        ok = (hin >= 0) & (hin < HIN)
        cc, oo = np.nonzero(ok)
        np.add.at(opv, (co_all[cc], k_all[cc] * HIN + hin[cc, oo], oo), w2r[g, cc])
    return opv, oph


def _identity_slots(w3, idx_identit):
    """Per block: 4 slots (k, coeff); distinct k's, zero-coeff fills."""
    w3r = np.asarray(w3, np.float32).reshape(G, C_OUT)
    idx = np.asarray(idx_identit, np.int64)       # (C_OUT, G)
    k_sel = idx - np.arange(C_OUT)[:, None] * NK
    assert np.all((k_sel >= 0) & (k_sel < NK))
    u = np.zeros((C_OUT, NK), np.float32)
    for g in range(G):
        np.add.at(u, (np.arange(C_OUT), k_sel[:, g]), w3r[g])
    ks = np.zeros((C_OUT, 4), np.int64)
    cf = np.zeros((C_OUT, 4), np.float32)
    for co in range(C_OUT):
        nz = list(np.nonzero(u[co])[0])
        fill = [k for k in range(NK) if k not in nz]
        kk = (nz + fill)[:4]
        ks[co] = kk
        cf[co, :len(nz)] = u[co, nz]
    return ks, cf


def _build_nc():
    import concourse.bacc as bacc
    import concourse.tile as tile
    import concourse.bass as bass
    import concourse.mybir as mybir
    from contextlib import ExitStack

    f32 = mybir.dt.float32
    f8 = mybir.dt.float8e3
    bf16 = mybir.dt.bfloat16

    nc = bacc.Bacc(None, target_bir_lowering=False)
    # main x per block: [p, orient, chunk(5), n]
    xall_d = nc.declare_dram_parameter(
        "xall", [BPC, KM, 2, NJM, NFREE], f8, isOutput=False)
    # operators per block: [p, bi, 672]: 5x112 V|H chunks then 2x56 identity
    opall_d = nc.declare_dram_parameter(
        "opall", [KM, BPC, OPW], f8, isOutput=False)
    # leftover x (11th channel group of both blocks): [pair, p, orient, n]
    xlft_d = nc.declare_dram_parameter(
        "xlft", [NPAIR, KM, 2, NFREE], f8, isOutput=False)
    # output: [p(120), pair, 3, 448] (cols: blk_e VH | blk_o VH | identity)
    out_d = nc.declare_dram_parameter(
        "out", [120, NPAIR, 3, NFREE], bf16, isOutput=True)

    with tile.TileContext(nc) as tc, ExitStack() as ctx:
        xpool = ctx.enter_context(tc.tile_pool(name="xp", bufs=1))
        oppool = ctx.enter_context(tc.tile_pool(name="opp", bufs=1))
        spool = ctx.enter_context(tc.tile_pool(name="stg", bufs=1))
        wpool = ctx.enter_context(tc.tile_pool(name="wp", bufs=1))
        psum_pool = ctx.enter_context(
            tc.tile_pool(name="psum", bufs=2, space=bass.MemorySpace.PSUM))
        wppool = ctx.enter_context(
            tc.tile_pool(name="wpp", bufs=1, space=bass.MemorySpace.PSUM))

        # ---- ring S (sync): tails+operators interleaved with per-block x;
        # ring A (scalar) idle during the input stream ----
        op_t = oppool.tile([KM, BPC, OPW], f8, tag="opall")
        stg = spool.tile([120, NPAIR, 3, NFREE], bf16, tag="stgall")
        x_ts = [None] * BPC
        lx_ts = []

        def load_x(bi):
            if bi == BPC - 1:
                xa = xpool.tile([KM, 2, 3, NFREE], f8, tag="xLa", name="xLa")
                xb = xpool.tile([KM, 2, 2, NFREE], f8, tag="xLb", name="xLb")
                nc.sync.dma_start(xa[:], xall_d[bi][:, :, 0:3])
                nc.sync.dma_start(xb[:], xall_d[bi][:, :, 3:NJM])
                x_ts[bi] = (xa, xb)
            else:
                x_t = xpool.tile([KM, 2, NJM, NFREE], f8, tag=f"x{bi}",
                                 name=f"x{bi}")
                nc.sync.dma_start(x_t[:], xall_d[bi])
                x_ts[bi] = x_t

        for q in range(NPAIR):
            nc.sync.dma_start(op_t[:, 2 * q:2 * q + 2],
                              opall_d[:, 2 * q:2 * q + 2])
            load_x(2 * q)
            load_x(2 * q + 1)
            lx_t = xpool.tile([KM, 2, NFREE], f8, tag=f"lx{q}", name=f"lx{q}")
            nc.sync.dma_start(lx_t[:], xlft_d[q])
            lx_ts.append(lx_t)

        # ---- PE warmup on memset tiles (no DMA dependency) ----
        warm = wpool.tile([KM, NFREE], f8, tag="warm")
        wst = wpool.tile([KM, 56], f8, tag="wst")
        nc.vector.memset(warm[:], 0)
        nc.vector.memset(wst[:], 0)
        pw = wppool.tile([128, NFREE], f32, tag="pw")
        for w in range(N_WARM):
            pos = (0, 0) if w % 2 == 0 else (0, 64)
            dst = pw[0:56] if w % 2 == 0 else pw[64:120]
            nc.tensor.matmul(dst, wst[:], warm[:], start=True, stop=True,
                             tile_position=pos)

        # ---- main: 6 block pairs ----
        for q in range(NPAIR):
            pvh = [psum_pool.tile([128, NFREE], f32, tag="pe", name=f"pe{q}"),
                   psum_pool.tile([128, NFREE], f32, tag="po", name=f"po{q}")]
            pi = psum_pool.tile([128, NFREE], f32, tag="pi", name=f"pi{q}")
            last = (q == NPAIR - 1)

            def xc(bi, o, j):
                t = x_ts[bi]
                if isinstance(t, tuple):
                    return t[0][:, o, j, :] if j < 3 else t[1][:, o, j - 3, :]
                return t[:, o, j, :]

            def vh(bi, pt, jlist, tail):
                for j in jlist:
                    nc.tensor.matmul(pt[0:56], op_t[:, bi, j * 112:j * 112 + 56],
                                     xc(bi, 0, j), start=(j == 0), stop=False,
                                     tile_position=(0, 0))
                    nc.tensor.matmul(pt[64:120],
                                     op_t[:, bi, j * 112 + 56:(j + 1) * 112],
                                     xc(bi, 1, j), start=(j == 0), stop=False,
                                     tile_position=(0, 64))
                if tail:
                    nc.tensor.matmul(pt[0:56], op_t[:, bi, 672:728],
                                     lx_ts[q][:, 0, :], start=False, stop=True,
                                     tile_position=(0, 0))
                    nc.tensor.matmul(pt[64:120], op_t[:, bi, 728:784],
                                     lx_ts[q][:, 1, :], start=False, stop=True,
                                     tile_position=(0, 64))

            def ident():
                for c in range(2):
                    o0 = NJM * 112 + c * 56
                    for b, colp, dst in ((0, 0, pi[0:56]), (1, 64, pi[64:120])):
                        bi = 2 * q + b
                        t = x_ts[bi]
                        xi = (t[0][0:120, 0, c, :] if isinstance(t, tuple)
                              else t[0:120, 0, c, :])
                        nc.tensor.matmul(dst, op_t[0:120, bi, o0:o0 + 56], xi,
                                         start=(c == 0), stop=(c == 1),
                                         tile_position=(0, colp))

            if last:
                # block e fully; block o through chunk 2; identity (needs
                # only chunks 0-1); then block o's final chunks + tails
                vh(2 * q, pvh[0], range(NJM), True)
                vh(2 * q + 1, pvh[1], range(3), False)
                ident()
                vh(2 * q + 1, pvh[1], range(3, NJM), True)
            else:
                vh(2 * q, pvh[0], range(NJM), True)
                vh(2 * q + 1, pvh[1], range(NJM), True)
                ident()
            # drain psums -> shared bf16 staging; flush at stream end
            if last:
                nc.vector.tensor_copy(stg[0:56, q, 2, :], pi[0:56])
                nc.scalar.copy(stg[64:120, q, 2, :], pi[64:120])
                nc.scalar.copy(stg[:, q, 0, :], pvh[0][0:120])
                nc.vector.tensor_copy(stg[:, q, 1, :], pvh[1][0:120])
            else:
                nc.scalar.copy(stg[:, q, 0, :], pvh[0][0:120])
                nc.vector.tensor_copy(stg[:, q, 1, :], pvh[1][0:120])
                nc.vector.tensor_copy(stg[0:56, q, 2, :], pi[0:56])
                nc.scalar.copy(stg[64:120, q, 2, :], pi[64:120])
        # pairs 0-4 as ONE 1.6 MB transfer, pair 5 split for early start
        nc.sync.dma_start(out_d[:, 0:NPAIR - 1], stg[:, 0:NPAIR - 1])
        nc.sync.dma_start(out_d[:, NPAIR - 1, 0:2], stg[:, NPAIR - 1, 0:2, :])
        nc.sync.dma_start(out_d[:, NPAIR - 1, 2], stg[:, NPAIR - 1, 2, :])
    nc.finalize()
    return nc


def prepare_inputs(x, w1, w2, w3, pad_hv, idx_identit):
    """Host-side shard prep. Returns in_maps (list of 8 dicts)."""
    x = np.asarray(x)
    xq = x.astype(F8)                                     # (B, C, 60, 60)
    opv, oph = _build_vh_operators(w1, w2, pad_hv)        # (96, 660, 56) f32
    ks, cf = _identity_slots(w3, idx_identit)             # (96,4) each
    eye = np.eye(56, dtype=np.float32)
    # spatial order inside each 64-row channel group: identity window first
    sp = np.concatenate([np.arange(2, 58), [0, 1, 58, 59]])  # (60,)

    in_maps = []
    for i in range(N_CORES):
        blocks = np.arange(i * BPC, (i + 1) * BPC)
        csl = slice(i * CPC, (i + 1) * CPC)
        # raw rows (k*60 + spatial): h-major and w-major
        ch = xq[:, csl, :, EP:EP + WOUT]                   # (8, 132, 60, 56)
        ch = ch.transpose(1, 2, 0, 3).reshape(BPC, NK * HIN, NFREE)
        cw = xq[:, csl, EP:EP + HOUT, :]                   # (8, 132, 56, 60)
        cw = cw.transpose(1, 3, 0, 2).reshape(BPC, NK * HIN, NFREE)
        opvh = np.concatenate([opv[blocks], oph[blocks]], axis=2)  # (12,660,112)
        # per-block row permutation: identity k's first, h-window order;
        # pad each group 60 -> 64 with zeros
        xnew = np.zeros((BPC, 2, KROWS, NFREE), np.float32)
        onew = np.zeros((BPC, KROWS, 112), np.float32)
        for bl in range(BPC):
            co = blocks[bl]
            others = [k for k in range(NK) if k not in ks[co]]
            k_order = list(ks[co]) + others
            rows = (np.asarray(k_order)[:, None] * HIN + sp[None, :]).ravel()
            src = np.stack([ch[bl], cw[bl]])               # (2, 660, 448)
            xnew[bl, :, :, :] = 0.0
            xnew[bl].reshape(2, NK, KG, NFREE)[:, :, :HIN] = (
                src[:, rows].reshape(2, NK, HIN, NFREE))
            onew[bl].reshape(NK, KG, 112)[:, :HIN] = (
                opvh[bl][rows].reshape(NK, HIN, 112))
        # main x: [bi, p, orient, chunk(5), n] from rows 0:640
        xm = (xnew[:, :, :NJM * KM].reshape(BPC, 2, NJM, KM, NFREE)
              .transpose(0, 3, 1, 2, 4))
        xall = np.ascontiguousarray(xm).astype(F8)
        # opall: [p, bi, 672]: main op chunks + identity bands
        opm = (onew[:, :NJM * KM].reshape(BPC, NJM, KM, 112)
               .transpose(2, 0, 1, 3).reshape(KM, BPC, NJM * 112))
        iop = np.zeros((KM, BPC, 2, 56), np.float32)
        for c in range(2):
            iop[0:56, :, c, :] = cf[blocks, 2 * c][None, :, None] * \
                eye[:, None, :]
            iop[64:120, :, c, :] = cf[blocks, 2 * c + 1][None, :, None] * \
                eye[:, None, :]
        lop = np.zeros((KM, BPC, 112), np.float32)
        for bl in range(BPC):
            r0 = KT * (bl % 2)
            lop[r0:r0 + KT, bl, :] = onew[bl, NJM * KM:]
        opall = np.ascontiguousarray(
            np.concatenate([opm, iop.reshape(KM, BPC, 112), lop], axis=2)
        ).astype(F8)                                       # (128, 12, 784)
        # leftover x: [pair, p, orient, n]; block e rows 0:64, o rows 64:128
        xlft = np.zeros((NPAIR, KM, 2, NFREE), np.float32)
        lt = xnew[:, :, NJM * KM:]                         # (12, 2, 64, 448)
        xlft[:, 0:KT] = lt[0::2].transpose(0, 2, 1, 3)
        xlft[:, KT:KM] = lt[1::2].transpose(0, 2, 1, 3)
        xlft = np.ascontiguousarray(xlft).astype(F8)
        in_maps.append({"xall": xall, "opall": opall, "xlft": xlft})
    return in_maps


def unshard(results):
    """-> (out_h, out_v, out_i) each (B, C_OUT, 56, 56) fp32."""
    o = np.stack([np.asarray(r["out"], np.float32) for r in results])
    o = o.transpose(0, 2, 1, 3, 4)             # -> (8, NPAIR, 120, 3, 448)
    # col 0 = blk_e, 1 = blk_o, 2 = identity
    vh = o[:, :, :, 0:2].transpose(0, 1, 3, 2, 4)  # (8, 6, 2, 120, 448)
    vh = vh.reshape(N_CORES, BPC, 120, NFREE)
    V = vh[:, :, 0:56].reshape(N_CORES, BPC, 56, B, WOUT)
    out_v = V.transpose(3, 0, 1, 2, 4).reshape(B, C_OUT, HOUT, WOUT)
    Hh = vh[:, :, 64:120].reshape(N_CORES, BPC, 56, B, HOUT)  # [.., w, b, h]
    out_h = Hh.transpose(3, 0, 1, 4, 2).reshape(B, C_OUT, HOUT, WOUT)
    ii = o[:, :, :, 2]                             # (8, 6, 120, 448)
    Ie = ii[:, :, 0:56].reshape(N_CORES, NPAIR, 56, B, WOUT)
    Io = ii[:, :, 64:120].reshape(N_CORES, NPAIR, 56, B, WOUT)
    I2 = np.stack([Ie, Io], axis=2)                # [core, pair, half, h, b, w]
    out_i = I2.transpose(4, 0, 1, 2, 3, 5).reshape(B, C_OUT, HOUT, WOUT)
    return out_h, out_v, out_i


def kernel(x, w1, w2, w3, pad_hv, idx_identit, b=B, hout=HOUT, wout=WOUT):
    from concourse.bass_utils import run_bass_kernel_spmd

    assert int(b) == B and int(hout) == HOUT and int(wout) == WOUT
    assert tuple(np.asarray(x).shape) == (B, C_IN, HIN, WIN)

    in_maps = prepare_inputs(x, w1, w2, w3, pad_hv, idx_identit)
    nc = _CACHE.get("nc")
    if nc is None:
        nc = _build_nc()
        _CACHE["nc"] = nc
    res = run_bass_kernel_spmd(nc, in_maps, core_ids=list(range(N_CORES)))
    return unshard(res.results)


# revision 38
# speedup vs baseline: 1.1333x; 1.1333x over previous
"""AddShift_mp_linear_module on 8 TRN2 NeuronCores — final.

Strategy (channel-block sharding, no collectives):
  - 96 output-channel blocks (11 input channels each) -> 12 blocks/core.
  - Per block, three dense contractions, all with M=56 output columns:
      V:  out_v[h, (b,w)]  over (k,h') rows   (h-major x)
      H:  out_hT[w, (b,h)] over (k,w') rows   (w-major x)
      I:  out_i[h, (b,w)]  reads 56-row slices of the SAME h-major x
    TWO chains run CONCURRENTLY on the PE via column tiling
    (tile_position (0,0)/(0,64)): measured pair span == single span
    (~190 ns warm, N=448), so PE work hides under the DMA stream.
  - Row layout per block: 11 channel groups of 64 rows (60 real + 4 pad),
    identity channels first with spatial order [2..57, 0, 1, 58, 59], so
    each identity window is a 0/64-aligned 56-row run readable straight
    from the main x tiles (no separate identity gather shipped). Identity
    = 2 K=120 matmuls/block with zero operator rows over the gaps.
  - 704 rows = 5x128 chunks + 64-row leftover; the two leftover groups of
    a block PAIR pack into one 128-partition chunk, consumed by per-block
    stationaries that are zero over the other block's rows. EVERY
    transfer is 128-partition (all 16 SDMA engines) -> ~410 GB/s.
  - DMA: ONE HWDGE ring (sync) in consumption order: per-pair operator
    slice, two x blocks, leftover chunk; outputs flush at the end from
    per-pair staging. ~10.7 MB/core total (the memory roofline).
  - Precision: fp8 e3m4 in, bf16 out (rel_err 1.85e-2 < 2e-2 gate).
Measured: ~41 us median (baseline 96.3 us).
"""

import numpy as np
import ml_dtypes

# architecture constants (match reference init_kwargs)
B = 8
C_OUT = 96
NK = 11
G = 4
C_IN = C_OUT * NK          # 1056
HOUT = WOUT = 56
HIN = WIN = 60
EP = 2                     # extra pad
N_CORES = 8
BPC = C_OUT // N_CORES     # blocks per core = 12
CPC = BPC * NK             # channels per core = 132
NPAIR = BPC // 2           # block pairs per core = 6
KG = 64                    # padded rows per channel group
KROWS = NK * KG            # 704 padded contraction rows per block
KM = 128                   # main chunk rows
NJM = 5                    # main chunks (5 x 128 = 640 rows = 10 groups)
KT = KG                    # tail chunk rows = 64 (the 11th group)
NFREE = B * WOUT           # 448 matmul free dim
N_WARM = 16                # PE warmup matmuls while first DMAs fly
OPW = NJM * 112 + 2 * 56 + 112  # operator cols per block = 784

F8 = ml_dtypes.float8_e3m4

_CACHE = {}


def _build_vh_operators(w1, w2, pad_hv):
    """Dense V/H operators per block: (C_OUT, 660, 56) fp32 each.
    Row r = k*60 + spatial_in for channel c = co*11 + k."""
    w1r = np.asarray(w1, np.float32).reshape(G, C_IN)
    w2r = np.asarray(w2, np.float32).reshape(G, C_IN)
    pad = np.asarray(pad_hv, np.int64)            # (C_IN, 2G)
    opv = np.zeros((C_OUT, NK * HIN, 56), np.float32)
    oph = np.zeros((C_OUT, NK * HIN, 56), np.float32)
    c_all = np.arange(C_IN)
    co_all = c_all // NK
    k_all = c_all % NK
    pos = np.arange(HOUT)
    for g in range(G):
        win = pos[None, :] + EP + pad[:, g][:, None]        # (C_IN, 56)
        ok = (win >= 0) & (win < WIN)
        cc, oo = np.nonzero(ok)
        np.add.at(oph, (co_all[cc], k_all[cc] * HIN + win[cc, oo], oo), w1r[g, cc])
        hin = pos[None, :] + EP + pad[:, G + g][:, None]
        ok = (hin >= 0) & (hin < HIN)
        cc, oo = np.nonzero(ok)
        np.add.at(opv, (co_all[cc], k_all[cc] * HIN + hin[cc, oo], oo), w2r[g, cc])
    return opv, oph


def _identity_slots(w3, idx_identit):
    """Per block: 4 slots (k, coeff); distinct k's, zero-coeff fills."""
    w3r = np.asarray(w3, np.float32).reshape(G, C_OUT)
    idx = np.asarray(idx_identit, np.int64)       # (C_OUT, G)
    k_sel = idx - np.arange(C_OUT)[:, None] * NK
    assert np.all((k_sel >= 0) & (k_sel < NK))
    u = np.zeros((C_OUT, NK), np.float32)
    for g in range(G):
        np.add.at(u, (np.arange(C_OUT), k_sel[:, g]), w3r[g])
    ks = np.zeros((C_OUT, 4), np.int64)
    cf = np.zeros((C_OUT, 4), np.float32)
    for co in range(C_OUT):
        nz = list(np.nonzero(u[co])[0])
        fill = [k for k in range(NK) if k not in nz]
        kk = (nz + fill)[:4]
        ks[co] = kk
        cf[co, :len(nz)] = u[co, nz]
    return ks, cf


def _build_nc():
    import concourse.bacc as bacc
    import concourse.tile as tile
    import concourse.bass as bass
    import concourse.mybir as mybir
    from contextlib import ExitStack

    f32 = mybir.dt.float32
    f8 = mybir.dt.float8e3
    bf16 = mybir.dt.bfloat16

    nc = bacc.Bacc(None, target_bir_lowering=False)
    # main x per block: [p, orient, chunk(5), n]
    xall_d = nc.declare_dram_parameter(
        "xall", [BPC, KM, 2, NJM, NFREE], f8, isOutput=False)
    # operators per block: [p, bi, 672]: 5x112 V|H chunks then 2x56 identity
    opall_d = nc.declare_dram_parameter(
        "opall", [KM, BPC, OPW], f8, isOutput=False)
    # leftover x (11th channel group of both blocks): [pair, p, orient, n]
    xlft_d = nc.declare_dram_parameter(
        "xlft", [NPAIR, KM, 2, NFREE], f8, isOutput=False)
    # output: per pair [120, 3, 448] (cols: blk_e VH | blk_o VH | identity)
    out_d = nc.declare_dram_parameter(
        "out", [NPAIR, 120, 3, NFREE], bf16, isOutput=True)

    with tile.TileContext(nc) as tc, ExitStack() as ctx:
        xpool = ctx.enter_context(tc.tile_pool(name="xp", bufs=1))
        oppool = ctx.enter_context(tc.tile_pool(name="opp", bufs=1))
        spool = ctx.enter_context(tc.tile_pool(name="stg", bufs=1))
        wpool = ctx.enter_context(tc.tile_pool(name="wp", bufs=1))
        psum_pool = ctx.enter_context(
            tc.tile_pool(name="psum", bufs=2, space=bass.MemorySpace.PSUM))
        wppool = ctx.enter_context(
            tc.tile_pool(name="wpp", bufs=1, space=bass.MemorySpace.PSUM))

        # ---- ring S (sync): tails+operators interleaved with per-block x;
        # ring A (scalar) idle during the input stream ----
        op_t = oppool.tile([KM, BPC, OPW], f8, tag="opall")
        x_ts = [None] * BPC
        lx_ts = []

        def load_x(bi):
            if bi == BPC - 1:
                xa = xpool.tile([KM, 2, 3, NFREE], f8, tag="xLa", name="xLa")
                xb = xpool.tile([KM, 2, 2, NFREE], f8, tag="xLb", name="xLb")
                nc.sync.dma_start(xa[:], xall_d[bi][:, :, 0:3])
                nc.sync.dma_start(xb[:], xall_d[bi][:, :, 3:NJM])
                x_ts[bi] = (xa, xb)
            else:
                x_t = xpool.tile([KM, 2, NJM, NFREE], f8, tag=f"x{bi}",
                                 name=f"x{bi}")
                nc.sync.dma_start(x_t[:], xall_d[bi])
                x_ts[bi] = x_t

        for q in range(NPAIR):
            nc.sync.dma_start(op_t[:, 2 * q:2 * q + 2],
                              opall_d[:, 2 * q:2 * q + 2])
            load_x(2 * q)
            load_x(2 * q + 1)
            lx_t = xpool.tile([KM, 2, NFREE], f8, tag=f"lx{q}", name=f"lx{q}")
            nc.sync.dma_start(lx_t[:], xlft_d[q])
            lx_ts.append(lx_t)

        # ---- PE warmup on memset tiles (no DMA dependency) ----
        warm = wpool.tile([KM, NFREE], f8, tag="warm")
        wst = wpool.tile([KM, 56], f8, tag="wst")
        nc.vector.memset(warm[:], 0)
        nc.vector.memset(wst[:], 0)
        pw = wppool.tile([128, NFREE], f32, tag="pw")
        for w in range(N_WARM):
            pos = (0, 0) if w % 2 == 0 else (0, 64)
            dst = pw[0:56] if w % 2 == 0 else pw[64:120]
            nc.tensor.matmul(dst, wst[:], warm[:], start=True, stop=True,
                             tile_position=pos)

        # ---- main: 6 block pairs ----
        out_stgs = []
        for q in range(NPAIR):
            pvh = [psum_pool.tile([128, NFREE], f32, tag="pe", name=f"pe{q}"),
                   psum_pool.tile([128, NFREE], f32, tag="po", name=f"po{q}")]
            pi = psum_pool.tile([128, NFREE], f32, tag="pi", name=f"pi{q}")
            last = (q == NPAIR - 1)

            def xc(bi, o, j):
                t = x_ts[bi]
                if isinstance(t, tuple):
                    return t[0][:, o, j, :] if j < 3 else t[1][:, o, j - 3, :]
                return t[:, o, j, :]

            def vh(bi, pt, jlist, tail):
                for j in jlist:
                    nc.tensor.matmul(pt[0:56], op_t[:, bi, j * 112:j * 112 + 56],
                                     xc(bi, 0, j), start=(j == 0), stop=False,
                                     tile_position=(0, 0))
                    nc.tensor.matmul(pt[64:120],
                                     op_t[:, bi, j * 112 + 56:(j + 1) * 112],
                                     xc(bi, 1, j), start=(j == 0), stop=False,
                                     tile_position=(0, 64))
                if tail:
                    nc.tensor.matmul(pt[0:56], op_t[:, bi, 672:728],
                                     lx_ts[q][:, 0, :], start=False, stop=True,
                                     tile_position=(0, 0))
                    nc.tensor.matmul(pt[64:120], op_t[:, bi, 728:784],
                                     lx_ts[q][:, 1, :], start=False, stop=True,
                                     tile_position=(0, 64))

            def ident():
                for c in range(2):
                    o0 = NJM * 112 + c * 56
                    for b, colp, dst in ((0, 0, pi[0:56]), (1, 64, pi[64:120])):
                        bi = 2 * q + b
                        t = x_ts[bi]
                        xi = (t[0][0:120, 0, c, :] if isinstance(t, tuple)
                              else t[0:120, 0, c, :])
                        nc.tensor.matmul(dst, op_t[0:120, bi, o0:o0 + 56], xi,
                                         start=(c == 0), stop=(c == 1),
                                         tile_position=(0, colp))

            if last:
                # block e fully; block o through chunk 2; identity (needs
                # only chunks 0-1); then block o's final chunks + tails
                vh(2 * q, pvh[0], range(NJM), True)
                vh(2 * q + 1, pvh[1], range(3), False)
                ident()
                vh(2 * q + 1, pvh[1], range(3, NJM), True)
            else:
                vh(2 * q, pvh[0], range(NJM), True)
                vh(2 * q + 1, pvh[1], range(NJM), True)
                ident()
            # drain psums -> bf16 staging; outputs flush at stream end
            stg = spool.tile([120, 3, NFREE], bf16, tag=f"stg{q}",
                             name=f"stg{q}")
            if last:
                nc.vector.tensor_copy(stg[0:56, 2, :], pi[0:56])
                nc.scalar.copy(stg[64:120, 2, :], pi[64:120])
                nc.scalar.copy(stg[:, 0, :], pvh[0][0:120])
                nc.vector.tensor_copy(stg[:, 1, :], pvh[1][0:120])
            else:
                nc.scalar.copy(stg[:, 0, :], pvh[0][0:120])
                nc.vector.tensor_copy(stg[:, 1, :], pvh[1][0:120])
                nc.vector.tensor_copy(stg[0:56, 2, :], pi[0:56])
                nc.scalar.copy(stg[64:120, 2, :], pi[64:120])
            out_stgs.append(stg)
        for q, stg in enumerate(out_stgs):
            if q == NPAIR - 1:
                nc.sync.dma_start(out_d[q, :, 0:2], stg[:, 0:2, :])
                nc.sync.dma_start(out_d[q, :, 2], stg[:, 2, :])
            else:
                nc.sync.dma_start(out_d[q], stg[:])
    nc.finalize()
    return nc


def prepare_inputs(x, w1, w2, w3, pad_hv, idx_identit):
    """Host-side shard prep. Returns in_maps (list of 8 dicts)."""
    x = np.asarray(x)
    xq = x.astype(F8)                                     # (B, C, 60, 60)
    opv, oph = _build_vh_operators(w1, w2, pad_hv)        # (96, 660, 56) f32
    ks, cf = _identity_slots(w3, idx_identit)             # (96,4) each
    eye = np.eye(56, dtype=np.float32)
    # spatial order inside each 64-row channel group: identity window first
    sp = np.concatenate([np.arange(2, 58), [0, 1, 58, 59]])  # (60,)

    in_maps = []
    for i in range(N_CORES):
        blocks = np.arange(i * BPC, (i + 1) * BPC)
        csl = slice(i * CPC, (i + 1) * CPC)
        # raw rows (k*60 + spatial): h-major and w-major
        ch = xq[:, csl, :, EP:EP + WOUT]                   # (8, 132, 60, 56)
        ch = ch.transpose(1, 2, 0, 3).reshape(BPC, NK * HIN, NFREE)
        cw = xq[:, csl, EP:EP + HOUT, :]                   # (8, 132, 56, 60)
        cw = cw.transpose(1, 3, 0, 2).reshape(BPC, NK * HIN, NFREE)
        opvh = np.concatenate([opv[blocks], oph[blocks]], axis=2)  # (12,660,112)
        # per-block row permutation: identity k's first, h-window order;
        # pad each group 60 -> 64 with zeros
        xnew = np.zeros((BPC, 2, KROWS, NFREE), np.float32)
        onew = np.zeros((BPC, KROWS, 112), np.float32)
        for bl in range(BPC):
            co = blocks[bl]
            others = [k for k in range(NK) if k not in ks[co]]
            k_order = list(ks[co]) + others
            rows = (np.asarray(k_order)[:, None] * HIN + sp[None, :]).ravel()
            src = np.stack([ch[bl], cw[bl]])               # (2, 660, 448)
            xnew[bl, :, :, :] = 0.0
            xnew[bl].reshape(2, NK, KG, NFREE)[:, :, :HIN] = (
                src[:, rows].reshape(2, NK, HIN, NFREE))
            onew[bl].reshape(NK, KG, 112)[:, :HIN] = (
                opvh[bl][rows].reshape(NK, HIN, 112))
        # main x: [bi, p, orient, chunk(5), n] from rows 0:640
        xm = (xnew[:, :, :NJM * KM].reshape(BPC, 2, NJM, KM, NFREE)
              .transpose(0, 3, 1, 2, 4))
        xall = np.ascontiguousarray(xm).astype(F8)
        # opall: [p, bi, 672]: main op chunks + identity bands
        opm = (onew[:, :NJM * KM].reshape(BPC, NJM, KM, 112)
               .transpose(2, 0, 1, 3).reshape(KM, BPC, NJM * 112))
        iop = np.zeros((KM, BPC, 2, 56), np.float32)
        for c in range(2):
            iop[0:56, :, c, :] = cf[blocks, 2 * c][None, :, None] * \
                eye[:, None, :]
            iop[64:120, :, c, :] = cf[blocks, 2 * c + 1][None, :, None] * \
                eye[:, None, :]
        lop = np.zeros((KM, BPC, 112), np.float32)
        for bl in range(BPC):
            r0 = KT * (bl % 2)
            lop[r0:r0 + KT, bl, :] = onew[bl, NJM * KM:]
        opall = np.ascontiguousarray(
            np.concatenate([opm, iop.reshape(KM, BPC, 112), lop], axis=2)
        ).astype(F8)                                       # (128, 12, 784)
        # leftover x: [pair, p, orient, n]; block e rows 0:64, o rows 64:128
        xlft = np.zeros((NPAIR, KM, 2, NFREE), np.float32)
        lt = xnew[:, :, NJM * KM:]                         # (12, 2, 64, 448)
        xlft[:, 0:KT] = lt[0::2].transpose(0, 2, 1, 3)
        xlft[:, KT:KM] = lt[1::2].transpose(0, 2, 1, 3)
        xlft = np.ascontiguousarray(xlft).astype(F8)
        in_maps.append({"xall": xall, "opall": opall, "xlft": xlft})
    return in_maps


def unshard(results):
    """-> (out_h, out_v, out_i) each (B, C_OUT, 56, 56) fp32."""
    o = np.stack([np.asarray(r["out"], np.float32) for r in results])
    # o: (8, NPAIR, 120, 3, 448); col 0 = blk_e, 1 = blk_o, 2 = identity
    vh = o[:, :, :, 0:2].transpose(0, 1, 3, 2, 4)  # (8, 6, 2, 120, 448)
    vh = vh.reshape(N_CORES, BPC, 120, NFREE)
    V = vh[:, :, 0:56].reshape(N_CORES, BPC, 56, B, WOUT)
    out_v = V.transpose(3, 0, 1, 2, 4).reshape(B, C_OUT, HOUT, WOUT)
    Hh = vh[:, :, 64:120].reshape(N_CORES, BPC, 56, B, HOUT)  # [.., w, b, h]
    out_h = Hh.transpose(3, 0, 1, 4, 2).reshape(B, C_OUT, HOUT, WOUT)
    ii = o[:, :, :, 2]                             # (8, 6, 120, 448)
    Ie = ii[:, :, 0:56].reshape(N_CORES, NPAIR, 56, B, WOUT)
    Io = ii[:, :, 64:120].reshape(N_CORES, NPAIR, 56, B, WOUT)
    I2 = np.stack([Ie, Io], axis=2)                # [core, pair, half, h, b, w]
    out_i = I2.transpose(4, 0, 1, 2, 3, 5).reshape(B, C_OUT, HOUT, WOUT)
    return out_h, out_v, out_i


def kernel(x, w1, w2, w3, pad_hv, idx_identit, b=B, hout=HOUT, wout=WOUT):
    from concourse.bass_utils import run_bass_kernel_spmd

    assert int(b) == B and int(hout) == HOUT and int(wout) == WOUT
    assert tuple(np.asarray(x).shape) == (B, C_IN, HIN, WIN)

    in_maps = prepare_inputs(x, w1, w2, w3, pad_hv, idx_identit)
    nc = _CACHE.get("nc")
    if nc is None:
        nc = _build_nc()
        _CACHE["nc"] = nc
    res = run_bass_kernel_spmd(nc, in_maps, core_ids=list(range(N_CORES)))
    return unshard(res.results)
